# revision 20
# baseline (speedup 1.0000x reference)
"""Two-layer single-head GAT (PyG GATConv semantics) on 8 Trainium2 NeuronCores.

v3 strategy (dst-sharded edge-parallel, slot-staged, bf16 tables):
  * Host: add self-loops, sort edges by destination, shard destinations
    across 8 cores (12500 each), pack each core's edges into 512-edge tiles
    (<=32 destination segments per tile, destinations consecutive).  Nodes
    are re-indexed into a uniform "slot" space (slot = tile*32 + seg, NSLOT
    = Tmax*32 identical on every core), so all per-tile addressing is
    compile-time static in the SPMD program; x is host-permuted to slot
    order.  GpSimd (SWDGE descriptor generation, ~8.6ns/descriptor) is the
    hard bottleneck, so only the per-edge source-row gathers (1 descriptor
    per edge per layer) plus a small final slot->node pass run there.
  * Phase 0: h1aug[slot] = x_slot @ [W1 | W1@as1 | W1@ad1] + [b1|0|0] (bias
    baked into rows: sum p*(h+b1)/sum p = agg+b1), stored bf16 with the
    a_dst column also written densely to adcol1; AllGather -> h1f.
  * Edge phase per layer / super-tile (8 tiles = 4096 edges): 32 indirect
    row gathers (128 edges x [feat | a_src] bf16 each); a_dst comes with
    ZERO per-edge descriptors: one static DMA reads adcol[S*256:(S+1)*256],
    a 1x128-ones matmul broadcasts it across partitions, and a one-hot
    multiply + axis-X tensor_reduce expands segment a_dst to edges.
    p = exp(leakyrelu(a_s + a_d)) scales the one-hot; per-tile matmul
    psA[seg, feat+1] = onehot^T @ rows puts segments on PSUM partitions so
    normalization is a per-partition tensor_scalar (fused relu for layer 1,
    then W2aug projection + [b2|0|0]); outputs are written to slot-ordered
    stage tables by static per-tile DMAs (no scatter descriptors).
  * AllGather of the layer-1 stage table feeds layer 2 (same gather slot
    indices); a final 98-call indirect pass converts the layer-2 stage to
    node order.
"""

import numpy as np
import ml_dtypes

BF16 = ml_dtypes.bfloat16

N_NODES = 100000
N_CORES = 8
F_IN, H, C = 128, 64, 40

TILE_E = 512            # edges per edge-tile
KPART = TILE_E // 128   # 4 edge chunks per tile
SEGCAP = 32             # destination-segment slots per edge-tile
SUPER = 8               # edge-tiles per super-tile
DUMMY_SEG = 99.0        # segid for padding edges: matches no one-hot column

DEF_CFG = dict(
    n=N_NODES, nshard=N_NODES // N_CORES, fin=F_IN, h=H, c=C,
    tile_e=TILE_E, k=KPART, segcap=SEGCAP, sup=SUPER,
)


# ----------------------------------------------------------------- host prep
def _pack_core(src_c, dst_c, base, cfg):
    nshard, tile_e, segcap = cfg["nshard"], cfg["tile_e"], cfg["segcap"]
    counts = np.bincount(dst_c - base, minlength=nshard)
    assert counts.min() >= 1 and counts.max() <= tile_e
    cum = np.concatenate([[0], np.cumsum(counts)])
    # tile slots: [0, segcap) one per destination seg (self-loop row, loaded
    # by a static DMA, not gathered), [segcap, tile_e) the remaining edges.
    tiles = []
    i = 0
    while i < nshard:
        j = int(np.searchsorted(cum, cum[i] + (tile_e - segcap),
                                side="right")) - 1
        while (j < nshard and j - i < segcap
               and cum[j + 1] - cum[i] <= tile_e - segcap + (j + 1 - i)):
            j += 1
        j = min(j, i + segcap, nshard)
        assert j > i
        tiles.append((i, j))
        i = j
    T = len(tiles)
    src_g = np.zeros((T, tile_e), np.int64)
    segid = np.full((T, tile_e), DUMMY_SEG, np.float32)
    nseg = np.zeros(T, np.int32)
    lo = np.zeros(T, np.int32)
    for t, (i, j) in enumerate(tiles):
        e0, e1 = int(cum[i]), int(cum[j])
        es = src_c[e0:e1]
        ed = dst_c[e0:e1]
        # one self-loop per destination goes to slot s; the rest after segcap
        sel = np.zeros(len(es), bool)
        selfish = np.where(es == ed)[0]
        first = {}
        for idx in selfish:
            d = int(ed[idx])
            if d not in first:
                first[d] = idx
                sel[idx] = True
        rest = np.where(~sel)[0]
        nn = j - i
        assert len(first) == nn, "missing self-loop for some destination"
        ds = np.array(sorted(first)) - base - i        # == arange(nn)
        src_g[t, :nn] = es[[first[base + i + s] for s in range(nn)]]
        segid[t, :nn] = np.arange(nn, dtype=np.float32)
        m = len(rest)
        assert segcap + m <= tile_e
        src_g[t, segcap:segcap + m] = es[rest]
        segid[t, segcap:segcap + m] = (ed[rest] - base - i).astype(np.float32)
        nseg[t] = nn
        lo[t] = i
    return src_g, segid, nseg, lo


def _edge_layout(arr_t):  # [T, tile_e] -> [128, T*K] with (t, k*128+p) -> [p, K*t+k]
    T, tile_e = arr_t.shape
    k = tile_e // 128
    return np.ascontiguousarray(
        arr_t.reshape(T, k, 128).transpose(2, 0, 1).reshape(128, T * k)
    )


def preprocess(edge_index, cfg):
    n, nshard, sup, segcap = cfg["n"], cfg["nshard"], cfg["sup"], cfg["segcap"]
    src = np.asarray(edge_index[0]).astype(np.int64)
    dst = np.asarray(edge_index[1]).astype(np.int64)
    loop = np.arange(n, dtype=np.int64)
    src = np.concatenate([src, loop])
    dst = np.concatenate([dst, loop])
    order = np.argsort(dst, kind="stable")
    src, dst = src[order], dst[order]
    bounds = np.searchsorted(dst, np.arange(N_CORES + 1) * nshard)
    packed = [
        _pack_core(src[bounds[c]: bounds[c + 1]], dst[bounds[c]: bounds[c + 1]],
                   c * nshard, cfg)
        for c in range(N_CORES)
    ]
    Tmax = max(p[0].shape[0] for p in packed)
    Tmax = -(-Tmax // sup) * sup
    nslot = Tmax * segcap
    # per-core slot maps: slot = t*segcap + s <-> local node lo[t]+s
    slotmaps = []
    slotnodes = []
    for c, (src_g, segid, nseg, lo) in enumerate(packed):
        T = len(nseg)
        slotmap = np.zeros(nshard, np.int64)
        slotnode = np.zeros(nslot, np.int64)  # local node of slot (pad -> 0)
        for t in range(T):
            s = np.arange(nseg[t])
            slotmap[lo[t] + s] = t * segcap + s
            slotnode[t * segcap + s] = lo[t] + s
        slotmaps.append(slotmap)
        slotnodes.append(slotnode)
    slotmap_all = np.stack(slotmaps)  # [cores, nshard]
    cores = []
    for c, (src_g, segid, nseg, lo) in enumerate(packed):
        pad = Tmax - src_g.shape[0]
        src_g = np.pad(src_g, ((0, pad), (0, 0)))
        segid = np.pad(segid, ((0, pad), (0, 0)), constant_values=DUMMY_SEG)
        # global slot index of each edge's source
        sc = src_g // nshard
        sl = slotmap_all[sc.ravel(), (src_g % nshard).ravel()].reshape(sc.shape)
        srcg = (sc * nslot + sl).astype(np.int32)
        # node -> slot conversion table for the final output pass
        ncall = -(-nshard // 128)
        outmap = np.zeros((128, ncall), np.int32)
        om = slotmaps[c]
        for j in range(ncall):
            w = min(128, nshard - j * 128)
            outmap[:w, j] = om[j * 128: j * 128 + w]
        srcg_l = _edge_layout(srcg)
        k = srcg.shape[1] // 128
        srcg0 = np.zeros((128, Tmax), np.int32)
        srcg0[0:96, :] = srcg_l[32:128, 0::k]
        cores.append(dict(
            srcg=srcg_l,
            srcg0=srcg0,
            segid=_edge_layout(segid).astype(BF16),
            outmap=outmap,
            slotnode=slotnodes[c],
        ))
    return cores, Tmax


def _compress_deps(nc):
    """Drop redundant sync dependencies so walrus' per-instruction HW wait
    slots don't overflow.  Producers on the same engine execute in issue
    order, and DMAs on the same logical queue complete in FIFO order, so a
    dependency on the latest producer of each stream subsumes the earlier
    ones.  Collectives are never dropped."""
    f = nc.m.functions[0]

    def all_insts(blk):
        for i in blk.instructions:
            yield i
        for sb in getattr(blk, "blocks", []) or []:
            yield from all_insts(sb)

    insts = [i for b in f.blocks for i in all_insts(b)]
    pos = {i.name: p for p, i in enumerate(insts)}
    by_name = {i.name: i for i in insts}

    def stream_key(p):
        tname = type(p).__name__
        if tname == "InstCollectiveCompute":
            return None  # own completion semaphore; never compress
        if tname == "InstDMACopy":
            return ("dma", str(getattr(p, "queue", "")), str(p.engine))
        return ("eng", str(p.engine))

    satisfied: dict = {}  # engine -> set of producer names already waited on
    for i in insts:
        deps = list(i.sync_dependency_names())
        eng = str(i.engine)
        sat = satisfied.setdefault(eng, set())
        if len(deps) > 2:
            best: dict = {}
            keep = []
            for d in deps:
                p = by_name.get(d)
                if p is None:
                    keep.append(d)
                    continue
                k = stream_key(p)
                if k is None:
                    keep.append(d)
                    continue
                cur = best.get(k)
                if cur is None or pos[d] > pos[cur]:
                    best[k] = d
            keep += list(best.values())
            for d in deps:
                if d not in keep:
                    i.try_remove_dependency(d)
            deps = keep
        # transitive pruning: same-engine instructions run in issue order, so
        # a dependency an earlier instruction on this engine already waited
        # for is satisfied for every later one.  Collectives stay explicit.
        for d in deps:
            p = by_name.get(d)
            if p is not None and type(p).__name__ == "InstCollectiveCompute":
                continue
            if d in sat:
                i.try_remove_dependency(d)
            else:
                sat.add(d)


# ------------------------------------------------------------- device program
def build_program(cfg, T):
    import concourse.bass as bass
    import concourse.bacc as bacc
    import concourse.mybir as mybir
    import concourse.tile as tile

    f32 = mybir.dt.float32
    bf = mybir.dt.bfloat16
    i32 = mybir.dt.int32
    AF = mybir.ActivationFunctionType
    AO = mybir.AluOpType
    AX = mybir.AxisListType
    nshard, fin, h, c = cfg["nshard"], cfg["fin"], cfg["h"], cfg["c"]
    k, segcap, sup = cfg["k"], cfg["segcap"], cfg["sup"]
    nsup = T // sup
    nslot = T * segcap
    scs = sup * segcap           # a_dst slots per super-tile (256)
    ncall = -(-nshard // 128)    # final conversion calls

    nc = bacc.Bacc(
        "TRN2", target_bir_lowering=False, debug=False,
        enable_asserts=False, num_devices=N_CORES,
    )

    xT = nc.dram_tensor("xT", [fin, nslot], f32, kind="ExternalInput").ap()
    w1aug = nc.dram_tensor("w1aug", [fin, h + 2], bf, kind="ExternalInput").ap()
    w2aug = nc.dram_tensor("w2aug", [h, c + 2], bf, kind="ExternalInput").ap()
    b1aug = nc.dram_tensor("b1aug", [h + 2, 1], f32, kind="ExternalInput").ap()
    b2aug = nc.dram_tensor("b2aug", [c + 2, 1], f32, kind="ExternalInput").ap()
    iota = nc.dram_tensor("iota", [128, segcap], bf, kind="ExternalInput").ap()
    ident = nc.dram_tensor("ident", [128, 128], bf, kind="ExternalInput").ap()
    srcg = nc.dram_tensor("srcg", [128, T * k], i32, kind="ExternalInput").ap()
    srcg0 = nc.dram_tensor("srcg0", [128, T], i32, kind="ExternalInput").ap()
    segid = nc.dram_tensor("segid", [128, T * k], bf, kind="ExternalInput").ap()
    outmap = nc.dram_tensor("outmap", [128, ncall], i32,
                            kind="ExternalInput").ap()
    out2 = nc.dram_tensor("out2", [nshard, c], f32, kind="ExternalOutput").ap()

    with tile.TileContext(nc) as tc:
        with (
            tc.tile_pool(name="consts", bufs=1) as cpool,
            tc.tile_pool(name="ph0", bufs=5) as ppool,
            tc.tile_pool(name="gath", bufs=6) as gpool,
            tc.tile_pool(name="epil", bufs=8) as epool,
            tc.tile_pool(name="psum", bufs=2, space="PSUM") as pp,
            tc.tile_pool(name="dram", bufs=1, space="DRAM") as dpool,
        ):
            # ---- constants
            w1aug_sb = cpool.tile([fin, h + 2], bf, name="w1aug_sb")
            nc.sync.dma_start(w1aug_sb[:], w1aug)
            w2aug_sb = cpool.tile([h, c + 2], bf, name="w2aug_sb")
            nc.sync.dma_start(w2aug_sb[:], w2aug)
            b1aug_sb = cpool.tile([h + 2, 1], f32, name="b1aug_sb")
            nc.sync.dma_start(b1aug_sb[:], b1aug)
            b2aug_sb = cpool.tile([c + 2, 1], f32, name="b2aug_sb")
            nc.sync.dma_start(b2aug_sb[:], b2aug)
            iota_sb = cpool.tile([128, segcap], bf, name="iota_sb")
            nc.sync.dma_start(iota_sb[:], iota)
            ident_sb = cpool.tile([128, 128], bf, name="ident_sb")
            nc.sync.dma_start(ident_sb[:], ident)
            ones_sb = cpool.tile([1, 128], bf, name="ones_sb")
            nc.vector.memset(ones_sb[:], 1.0)
            srcg_sb = cpool.tile([128, T * k], i32, name="srcg_sb")
            nc.sync.dma_start(srcg_sb[:], srcg)
            srcg0_sb = cpool.tile([128, T], i32, name="srcg0_sb")
            nc.sync.dma_start(srcg0_sb[:], srcg0)
            segid_sb = cpool.tile([128, T * k], bf, name="segid_sb")
            nc.sync.dma_start(segid_sb[:], segid)
            outmap_sb = cpool.tile([128, ncall], i32, name="outmap_sb")
            nc.sync.dma_start(outmap_sb[:], outmap)

            # ---- internal DRAM (slot-ordered stages)
            h1s = dpool.tile([nslot, h + 2], bf, name="h1s")
            h1f = dpool.tile([N_CORES * nslot, h + 2], bf, name="h1f",
                             addr_space="Shared")
            g2s = dpool.tile([nslot, c + 2], bf, name="g2s")
            g2f = dpool.tile([N_CORES * nslot, c + 2], bf, name="g2f",
                             addr_space="Shared")
            fins = dpool.tile([nslot, c], f32, name="fins")

            # ---- phase 0: h1aug by slot, + dense a_dst column
            Hh = nslot // 2
            PH = 512
            for o in range(0, nslot, PH):
                w = min(PH, nslot - o)
                xt = ppool.tile([fin, PH], f32, name="xt")
                nc.sync.dma_start(xt[:, 0:w], xT[:, o:o + w])
                xtb = ppool.tile([fin, PH], bf, name="xtb")
                nc.vector.tensor_copy(xtb[:, 0:w], xt[:, 0:w])
                psH = pp.tile([h + 2, PH], f32, name="psH", tag="pA")
                nc.tensor.matmul(psH[:, 0:w], lhsT=w1aug_sb[:],
                                 rhs=xtb[:, 0:w], start=True, stop=True)
                h1t = ppool.tile([h + 2, PH], bf, name="h1t")
                nc.scalar.activation(h1t[:, 0:w], psH[:, 0:w], AF.Identity,
                                     bias=b1aug_sb[:])
                for q in range(0, w, 128):
                    wq = min(128, w - q)
                    psT0 = pp.tile([128, h + 2], bf, name="psT0",
                                   tag="pT" if (q // 128) % 2 == 0 else "pC")
                    nc.tensor.transpose(
                        psT0[0:wq, :], in_=h1t[:, q:q + wq],
                        identity=ident_sb[0:h + 2, 0:h + 2])
                    h1r = ppool.tile([128, h + 2], bf, name="h1r")
                    nc.vector.tensor_copy(h1r[0:wq, :], psT0[0:wq, :])
                    nc.sync.dma_start(h1s[o + q:o + q + wq, :], h1r[0:wq, :])

            nc.gpsimd.collective_compute(
                "AllGather", mybir.AluOpType.bypass,
                replica_groups=[list(range(N_CORES))],
                ins=[h1s[:]], outs=[h1f[:]],
            )

            # ---- edge phases
            def edge_layer(table, fdim, stage, last, hook=None):
                """gathers rw = fdim+1 elems [feat | a_src] per edge from
                table [8*nslot, fdim+2]; a_dst comes from the local stage
                table's a_dst column via a strided per-super DMA."""
                rw = fdim + 1
                for S in range(nsup):
                    c0 = sup * k * S
                    rows = gpool.tile([128, sup * k * rw], bf,
                                      name=f"rows{last}")
                    for ch in range(sup * k):
                        if ch % k == 0:
                            t = S * sup + ch // k
                            nc.gpsimd.indirect_dma_start(
                                out=rows[segcap:128, ch * rw:(ch + 1) * rw],
                                out_offset=None, in_=table[:],
                                in_offset=bass.IndirectOffsetOnAxis(
                                    ap=srcg0_sb[0:128 - segcap, t:t + 1],
                                    axis=0),
                                element_offset=0,
                            )
                        else:
                            nc.gpsimd.indirect_dma_start(
                                out=rows[:, ch * rw:(ch + 1) * rw],
                                out_offset=None, in_=table[:],
                                in_offset=bass.IndirectOffsetOnAxis(
                                    ap=srcg_sb[:, c0 + ch:c0 + ch + 1],
                                    axis=0),
                                element_offset=0,
                            )
                    for i in range(sup):
                        # chunk-0 partitions [0, segcap): self-loop rows are
                        # contiguous rows of the LOCAL stage at a static offset
                        t = S * sup + i
                        nc.sync.dma_start(
                            rows[0:segcap, i * k * rw:i * k * rw + rw],
                            stage[t * segcap:(t + 1) * segcap, 0:rw])
                    # broadcast this super-tile's a_dst slots to all partitions
                    adrow = gpool.tile([1, scs], bf, name=f"adrow{last}")
                    nc.sync.dma_start(
                        adrow[:],
                        stage[S * scs:(S + 1) * scs,
                              fdim + 1:fdim + 2].transpose([1, 0]))
                    psB = pp.tile([128, scs], f32, name=f"psB{last}",
                                   tag="pC")
                    nc.tensor.matmul(psB[:], lhsT=ones_sb[:],
                                     rhs=adrow[:], start=True, stop=True)
                    adflat = gpool.tile([128, scs], bf, name=f"adf{last}")
                    nc.vector.tensor_copy(adflat[:], psB[:])
                    # one-hot of segids, a_dst expansion, attention weights
                    ohs = gpool.tile([128, sup * k * segcap], bf,
                                     name=f"ohs{last}")
                    ov = ohs[:].rearrange("p (e s) -> p e s", s=segcap)
                    nc.vector.tensor_tensor(
                        out=ov,
                        in0=iota_sb[:].unsqueeze(1).broadcast_to(
                            [128, sup * k, segcap]),
                        in1=segid_sb[:, c0:c0 + sup * k].unsqueeze(
                            2).broadcast_to([128, sup * k, segcap]),
                        op=AO.is_equal)
                    tmp = gpool.tile([128, sup * k * segcap], bf,
                                     name=f"tmp{last}")
                    nc.vector.tensor_tensor(
                        out=tmp[:].rearrange("p (t q s) -> p t q s", q=k,
                                             s=segcap),
                        in0=ohs[:].rearrange("p (t q s) -> p t q s", q=k,
                                             s=segcap),
                        in1=adflat[:].rearrange(
                            "p (t s) -> p t s", s=segcap).unsqueeze(
                            2).broadcast_to([128, sup, k, segcap]),
                        op=AO.mult)
                    ade = gpool.tile([128, sup * k], f32, name=f"ade{last}")
                    nc.vector.tensor_reduce(
                        ade[:], tmp[:].rearrange("p (e s) -> p e s",
                                                 s=segcap),
                        axis=AX.X, op=AO.add)
                    rv = rows[:].rearrange("p (e f) -> p e f", f=rw)
                    es = gpool.tile([128, sup * k], f32, name=f"es{last}")
                    nc.vector.tensor_tensor(out=es[:], in0=rv[:, :, rw - 1],
                                            in1=ade[:], op=AO.add)
                    e2 = gpool.tile([128, sup * k], f32, name=f"e2{last}")
                    nc.vector.tensor_scalar_mul(e2[:], es[:], 0.2)
                    nc.vector.tensor_tensor(out=es[:], in0=es[:], in1=e2[:],
                                            op=AO.max)
                    ps = gpool.tile([128, sup * k], bf, name=f"ps{last}")
                    nc.scalar.activation(ps[:], es[:], AF.Exp)
                    nc.vector.memset(rv[:, :, rw - 1], 1.0)
                    nc.vector.tensor_tensor(
                        out=ov, in0=ov,
                        in1=ps[:].unsqueeze(2).broadcast_to(
                            [128, sup * k, segcap]),
                        op=AO.mult)
                    for i in range(sup):
                        t = S * sup + i
                        psA = pp.tile([segcap, rw], f32, name=f"psA{last}",
                                      tag="pA")
                        for kk in range(k):
                            ch = i * k + kk
                            nc.tensor.matmul(
                                psA[:],
                                lhsT=ohs[:, ch * segcap:(ch + 1) * segcap],
                                rhs=rows[:, ch * rw:(ch + 1) * rw],
                                start=(kk == 0), stop=(kk == k - 1))
                        den = epool.tile([segcap, 1], f32, name=f"den{last}")
                        nc.vector.tensor_scalar(
                            den[:], psA[:, fdim:fdim + 1], 1e-30, None,
                            op0=AO.max)
                        rcp = epool.tile([segcap, 1], f32, name=f"rcp{last}")
                        nc.vector.reciprocal(rcp[:], den[:])
                        if not last:
                            h2n = epool.tile([segcap, fdim], bf, name="h2n")
                            nc.vector.tensor_scalar(
                                h2n[:], psA[:, 0:fdim], rcp[:], 0.0,
                                op0=AO.mult, op1=AO.max)
                            psT = pp.tile([fdim, segcap], bf, name="psT",
                                          tag="pT")
                            nc.tensor.transpose(
                                psT[:], in_=h2n[:],
                                identity=ident_sb[0:segcap, 0:segcap])
                            h2nT = epool.tile([fdim, segcap], bf, name="h2nT")
                            nc.vector.tensor_copy(h2nT[:], psT[:])
                            psC = pp.tile([c + 2, segcap], f32, name="psC",
                                          tag="pC")
                            nc.tensor.matmul(psC[:], lhsT=w2aug_sb[:],
                                             rhs=h2nT[:], start=True,
                                             stop=True)
                            c1 = epool.tile([c + 2, segcap], bf, name="c1")
                            nc.scalar.activation(c1[:], psC[:], AF.Identity,
                                                 bias=b2aug_sb[:])
                            psD = pp.tile([segcap, c + 2], bf, name="psD",
                                          tag="pT")
                            nc.tensor.transpose(
                                psD[:], in_=c1[:],
                                identity=ident_sb[0:c + 2, 0:c + 2])
                            orow = epool.tile([segcap, c + 2], bf,
                                              name="orow")
                            nc.vector.tensor_copy(orow[:], psD[:])
                            nc.sync.dma_start(
                                g2s[t * segcap:(t + 1) * segcap, :], orow[:])
                        else:
                            fin_t = epool.tile([segcap, c], f32, name="fin")
                            nc.vector.tensor_scalar(
                                fin_t[:], psA[:, 0:c], rcp[:], None,
                                op0=AO.mult)
                            nc.sync.dma_start(
                                fins[t * segcap:(t + 1) * segcap, :],
                                fin_t[:])
                    if hook is not None:
                        hook(S)

            edge_layer(h1f, h, h1s, last=False)
            nc.gpsimd.collective_compute(
                "AllGather", mybir.AluOpType.bypass,
                replica_groups=[list(range(N_CORES))],
                ins=[g2s[:]], outs=[g2f[:]],
            )

            # ---- slot -> node conversion, interleaved into the L2 loop.
            # slotmap is monotone and slots >= nodes, so the slots of nodes
            # [0, 128(j+1)) all lie below nslot - nshard + 128(j+1): call j
            # may run once the stage rows below that bound are final.
            conv_done = [0]

            def emit_conv(upto):
                while conv_done[0] < ncall:
                    j = conv_done[0]
                    bound = min(nslot, nslot - nshard + 128 * (j + 1))
                    if bound > upto:
                        break
                    w = min(128, nshard - j * 128)
                    fj = epool.tile([128, c], f32, name="fj")
                    nc.gpsimd.indirect_dma_start(
                        out=fj[0:w, :], out_offset=None,
                        in_=fins[0:bound, :],
                        in_offset=bass.IndirectOffsetOnAxis(
                            ap=outmap_sb[0:w, j:j + 1], axis=0),
                        element_offset=0,
                    )
                    nc.sync.dma_start(out2[j * 128:j * 128 + w, :],
                                      fj[0:w, :])
                    conv_done[0] += 1

            def l2_hook(S):
                emit_conv((S + 1) * sup * segcap)

            edge_layer(g2f, c, g2s, last=True, hook=l2_hook)
            emit_conv(nslot)

    _compress_deps(nc)
    nc.compile()
    return nc


# ------------------------------------------------------------------ interface
def make_inmaps(inputs, cfg):
    x = np.ascontiguousarray(np.asarray(inputs["x"], np.float32))
    W1 = np.asarray(inputs["W1"], np.float32)
    as1 = np.asarray(inputs["att_src1"], np.float32)
    ad1 = np.asarray(inputs["att_dst1"], np.float32)
    b1 = np.asarray(inputs["b1"], np.float32)
    W2 = np.asarray(inputs["W2"], np.float32)
    as2 = np.asarray(inputs["att_src2"], np.float32)
    ad2 = np.asarray(inputs["att_dst2"], np.float32)
    b2 = np.asarray(inputs["b2"], np.float32)
    cores, T = preprocess(np.asarray(inputs["edge_index"]), cfg)
    w1aug = np.concatenate([W1, (W1 @ as1)[:, None], (W1 @ ad1)[:, None]], 1)
    w2aug = np.concatenate([W2, (W2 @ as2)[:, None], (W2 @ ad2)[:, None]], 1)
    b1aug = np.concatenate([b1, [0.0, 0.0]]).astype(np.float32)[:, None]
    b2aug = np.concatenate([b2, [0.0, 0.0]]).astype(np.float32)[:, None]
    nshard, segcap = cfg["nshard"], cfg["segcap"]
    iota = np.broadcast_to(np.arange(segcap, dtype=np.float32),
                           (128, segcap)).astype(BF16)
    ident = np.eye(128, dtype=np.float32).astype(BF16)
    in_maps = []
    for cidx in range(N_CORES):
        xs = x[cidx * nshard:(cidx + 1) * nshard]      # [nshard, fin]
        xslot = xs[cores[cidx]["slotnode"]]            # [nslot, fin]
        in_maps.append(dict(
            xT=np.ascontiguousarray(xslot.T),
            w1aug=np.ascontiguousarray(w1aug.astype(BF16)),
            w2aug=np.ascontiguousarray(w2aug.astype(BF16)),
            b1aug=np.ascontiguousarray(b1aug),
            b2aug=np.ascontiguousarray(b2aug),
            iota=np.ascontiguousarray(iota),
            ident=np.ascontiguousarray(ident),
            srcg=cores[cidx]["srcg"],
            srcg0=cores[cidx]["srcg0"],
            segid=cores[cidx]["segid"],
            outmap=cores[cidx]["outmap"],
        ))
    return in_maps, T


def kernel(**inputs):
    from concourse import bass_utils

    cfg = dict(DEF_CFG)
    in_maps, T = make_inmaps(inputs, cfg)
    nc = build_program(cfg, T)
    res = bass_utils.run_bass_kernel_spmd(
        nc, in_maps, core_ids=list(range(N_CORES)))
    out = np.concatenate([res.results[c]["out2"] for c in range(N_CORES)], 0)
    return out.astype(np.float32)


# revision 21
# speedup vs baseline: 1.0048x; 1.0048x over previous
"""Two-layer single-head GAT (PyG GATConv semantics) on 8 Trainium2 NeuronCores.

v3 strategy (dst-sharded edge-parallel, slot-staged, bf16 tables):
  * Host: add self-loops, sort edges by destination, shard destinations
    across 8 cores (12500 each), pack each core's edges into 512-edge tiles
    (<=32 destination segments per tile, destinations consecutive).  Nodes
    are re-indexed into a uniform "slot" space (slot = tile*32 + seg, NSLOT
    = Tmax*32 identical on every core), so all per-tile addressing is
    compile-time static in the SPMD program; x is host-permuted to slot
    order.  GpSimd (SWDGE descriptor generation, ~8.6ns/descriptor) is the
    hard bottleneck, so only the per-edge source-row gathers (1 descriptor
    per edge per layer) plus a small final slot->node pass run there.
  * Phase 0: h1aug[slot] = x_slot @ [W1 | W1@as1 | W1@ad1] + [b1|0|0] (bias
    baked into rows: sum p*(h+b1)/sum p = agg+b1), stored bf16 with the
    a_dst column also written densely to adcol1; AllGather -> h1f.
  * Edge phase per layer / super-tile (8 tiles = 4096 edges): 32 indirect
    row gathers (128 edges x [feat | a_src] bf16 each); a_dst comes with
    ZERO per-edge descriptors: one static DMA reads adcol[S*256:(S+1)*256],
    a 1x128-ones matmul broadcasts it across partitions, and a one-hot
    multiply + axis-X tensor_reduce expands segment a_dst to edges.
    p = exp(leakyrelu(a_s + a_d)) scales the one-hot; per-tile matmul
    psA[seg, feat+1] = onehot^T @ rows puts segments on PSUM partitions so
    normalization is a per-partition tensor_scalar (fused relu for layer 1,
    then W2aug projection + [b2|0|0]); outputs are written to slot-ordered
    stage tables by static per-tile DMAs (no scatter descriptors).
  * AllGather of the layer-1 stage table feeds layer 2 (same gather slot
    indices); a final 98-call indirect pass converts the layer-2 stage to
    node order.
"""

import numpy as np
import ml_dtypes

BF16 = ml_dtypes.bfloat16

N_NODES = 100000
N_CORES = 8
F_IN, H, C = 128, 64, 40

TILE_E = 512            # edges per edge-tile
KPART = TILE_E // 128   # 4 edge chunks per tile
SEGCAP = 32             # destination-segment slots per edge-tile
SUPER = 8               # edge-tiles per super-tile
DUMMY_SEG = 99.0        # segid for padding edges: matches no one-hot column

DEF_CFG = dict(
    n=N_NODES, nshard=N_NODES // N_CORES, fin=F_IN, h=H, c=C,
    tile_e=TILE_E, k=KPART, segcap=SEGCAP, sup=SUPER,
)


# ----------------------------------------------------------------- host prep
def _pack_core(src_c, dst_c, base, cfg):
    nshard, tile_e, segcap = cfg["nshard"], cfg["tile_e"], cfg["segcap"]
    counts = np.bincount(dst_c - base, minlength=nshard)
    assert counts.min() >= 1 and counts.max() <= tile_e
    cum = np.concatenate([[0], np.cumsum(counts)])
    # tile slots: [0, segcap) one per destination seg (self-loop row, loaded
    # by a static DMA, not gathered), [segcap, tile_e) the remaining edges.
    tiles = []
    i = 0
    while i < nshard:
        j = int(np.searchsorted(cum, cum[i] + (tile_e - segcap),
                                side="right")) - 1
        while (j < nshard and j - i < segcap
               and cum[j + 1] - cum[i] <= tile_e - segcap + (j + 1 - i)):
            j += 1
        j = min(j, i + segcap, nshard)
        assert j > i
        tiles.append((i, j))
        i = j
    T = len(tiles)
    src_g = np.zeros((T, tile_e), np.int64)
    segid = np.full((T, tile_e), DUMMY_SEG, np.float32)
    nseg = np.zeros(T, np.int32)
    lo = np.zeros(T, np.int32)
    for t, (i, j) in enumerate(tiles):
        e0, e1 = int(cum[i]), int(cum[j])
        es = src_c[e0:e1]
        ed = dst_c[e0:e1]
        # one self-loop per destination goes to slot s; the rest after segcap
        sel = np.zeros(len(es), bool)
        selfish = np.where(es == ed)[0]
        first = {}
        for idx in selfish:
            d = int(ed[idx])
            if d not in first:
                first[d] = idx
                sel[idx] = True
        rest = np.where(~sel)[0]
        nn = j - i
        assert len(first) == nn, "missing self-loop for some destination"
        ds = np.array(sorted(first)) - base - i        # == arange(nn)
        src_g[t, :nn] = es[[first[base + i + s] for s in range(nn)]]
        segid[t, :nn] = np.arange(nn, dtype=np.float32)
        m = len(rest)
        assert segcap + m <= tile_e
        src_g[t, segcap:segcap + m] = es[rest]
        segid[t, segcap:segcap + m] = (ed[rest] - base - i).astype(np.float32)
        nseg[t] = nn
        lo[t] = i
    return src_g, segid, nseg, lo


def _edge_layout(arr_t):  # [T, tile_e] -> [128, T*K] with (t, k*128+p) -> [p, K*t+k]
    T, tile_e = arr_t.shape
    k = tile_e // 128
    return np.ascontiguousarray(
        arr_t.reshape(T, k, 128).transpose(2, 0, 1).reshape(128, T * k)
    )


def preprocess(edge_index, cfg):
    n, nshard, sup, segcap = cfg["n"], cfg["nshard"], cfg["sup"], cfg["segcap"]
    src = np.asarray(edge_index[0]).astype(np.int64)
    dst = np.asarray(edge_index[1]).astype(np.int64)
    loop = np.arange(n, dtype=np.int64)
    src = np.concatenate([src, loop])
    dst = np.concatenate([dst, loop])
    order = np.argsort(dst, kind="stable")
    src, dst = src[order], dst[order]
    bounds = np.searchsorted(dst, np.arange(N_CORES + 1) * nshard)
    packed = [
        _pack_core(src[bounds[c]: bounds[c + 1]], dst[bounds[c]: bounds[c + 1]],
                   c * nshard, cfg)
        for c in range(N_CORES)
    ]
    Tmax = max(p[0].shape[0] for p in packed)
    Tmax = -(-Tmax // sup) * sup
    nslot = Tmax * segcap
    # per-core slot maps: slot = t*segcap + s <-> local node lo[t]+s
    slotmaps = []
    slotnodes = []
    for c, (src_g, segid, nseg, lo) in enumerate(packed):
        T = len(nseg)
        slotmap = np.zeros(nshard, np.int64)
        slotnode = np.zeros(nslot, np.int64)  # local node of slot (pad -> 0)
        for t in range(T):
            s = np.arange(nseg[t])
            slotmap[lo[t] + s] = t * segcap + s
            slotnode[t * segcap + s] = lo[t] + s
        slotmaps.append(slotmap)
        slotnodes.append(slotnode)
    slotmap_all = np.stack(slotmaps)  # [cores, nshard]
    cores = []
    for c, (src_g, segid, nseg, lo) in enumerate(packed):
        pad = Tmax - src_g.shape[0]
        src_g = np.pad(src_g, ((0, pad), (0, 0)))
        segid = np.pad(segid, ((0, pad), (0, 0)), constant_values=DUMMY_SEG)
        # global slot index of each edge's source
        sc = src_g // nshard
        sl = slotmap_all[sc.ravel(), (src_g % nshard).ravel()].reshape(sc.shape)
        srcg = (sc * nslot + sl).astype(np.int32)
        # node -> slot conversion table for the final output pass
        ncall = -(-nshard // 128)
        outmap = np.zeros((128, ncall), np.int32)
        om = slotmaps[c]
        for j in range(ncall):
            w = min(128, nshard - j * 128)
            outmap[:w, j] = om[j * 128: j * 128 + w]
        srcg_l = _edge_layout(srcg)
        k = srcg.shape[1] // 128
        srcg0 = np.zeros((128, Tmax), np.int32)
        srcg0[0:96, :] = srcg_l[32:128, 0::k]
        cores.append(dict(
            srcg=srcg_l,
            srcg0=srcg0,
            segid=_edge_layout(segid).astype(BF16),
            outmap=outmap,
            slotnode=slotnodes[c],
        ))
    return cores, Tmax


def _compress_deps(nc):
    """Drop redundant sync dependencies so walrus' per-instruction HW wait
    slots don't overflow.  Producers on the same engine execute in issue
    order, and DMAs on the same logical queue complete in FIFO order, so a
    dependency on the latest producer of each stream subsumes the earlier
    ones.  Collectives are never dropped."""
    f = nc.m.functions[0]

    def all_insts(blk):
        for i in blk.instructions:
            yield i
        for sb in getattr(blk, "blocks", []) or []:
            yield from all_insts(sb)

    insts = [i for b in f.blocks for i in all_insts(b)]
    pos = {i.name: p for p, i in enumerate(insts)}
    by_name = {i.name: i for i in insts}

    def stream_key(p):
        tname = type(p).__name__
        if tname == "InstCollectiveCompute":
            return None  # own completion semaphore; never compress
        if tname == "InstDMACopy":
            return ("dma", str(getattr(p, "queue", "")), str(p.engine))
        return ("eng", str(p.engine))

    satisfied: dict = {}  # engine -> set of producer names already waited on
    for i in insts:
        deps = list(i.sync_dependency_names())
        eng = str(i.engine)
        sat = satisfied.setdefault(eng, set())
        if len(deps) > 2:
            best: dict = {}
            keep = []
            for d in deps:
                p = by_name.get(d)
                if p is None:
                    keep.append(d)
                    continue
                k = stream_key(p)
                if k is None:
                    keep.append(d)
                    continue
                cur = best.get(k)
                if cur is None or pos[d] > pos[cur]:
                    best[k] = d
            keep += list(best.values())
            for d in deps:
                if d not in keep:
                    i.try_remove_dependency(d)
            deps = keep
        # transitive pruning: same-engine instructions run in issue order, so
        # a dependency an earlier instruction on this engine already waited
        # for is satisfied for every later one.  Collectives stay explicit.
        for d in deps:
            p = by_name.get(d)
            if p is not None and type(p).__name__ == "InstCollectiveCompute":
                continue
            if d in sat:
                i.try_remove_dependency(d)
            else:
                sat.add(d)


# ------------------------------------------------------------- device program
def build_program(cfg, T):
    import concourse.bass as bass
    import concourse.bacc as bacc
    import concourse.mybir as mybir
    import concourse.tile as tile

    f32 = mybir.dt.float32
    bf = mybir.dt.bfloat16
    i32 = mybir.dt.int32
    AF = mybir.ActivationFunctionType
    AO = mybir.AluOpType
    AX = mybir.AxisListType
    nshard, fin, h, c = cfg["nshard"], cfg["fin"], cfg["h"], cfg["c"]
    k, segcap, sup = cfg["k"], cfg["segcap"], cfg["sup"]
    nsup = T // sup
    nslot = T * segcap
    scs = sup * segcap           # a_dst slots per super-tile (256)
    ncall = -(-nshard // 128)    # final conversion calls

    nc = bacc.Bacc(
        "TRN2", target_bir_lowering=False, debug=False,
        enable_asserts=False, num_devices=N_CORES,
    )

    xT = nc.dram_tensor("xT", [fin, nslot], f32, kind="ExternalInput").ap()
    w1aug = nc.dram_tensor("w1aug", [fin, h + 2], bf, kind="ExternalInput").ap()
    w2aug = nc.dram_tensor("w2aug", [h, c + 2], bf, kind="ExternalInput").ap()
    b1aug = nc.dram_tensor("b1aug", [h + 2, 1], f32, kind="ExternalInput").ap()
    b2aug = nc.dram_tensor("b2aug", [c + 2, 1], f32, kind="ExternalInput").ap()
    iota = nc.dram_tensor("iota", [128, segcap], bf, kind="ExternalInput").ap()
    ident = nc.dram_tensor("ident", [128, 128], bf, kind="ExternalInput").ap()
    srcg = nc.dram_tensor("srcg", [128, T * k], i32, kind="ExternalInput").ap()
    srcg0 = nc.dram_tensor("srcg0", [128, T], i32, kind="ExternalInput").ap()
    segid = nc.dram_tensor("segid", [128, T * k], bf, kind="ExternalInput").ap()
    outmap = nc.dram_tensor("outmap", [128, ncall], i32,
                            kind="ExternalInput").ap()
    out2 = nc.dram_tensor("out2", [nshard, c], f32, kind="ExternalOutput").ap()

    with tile.TileContext(nc) as tc:
        with (
            tc.tile_pool(name="consts", bufs=1) as cpool,
            tc.tile_pool(name="ph0", bufs=5) as ppool,
            tc.tile_pool(name="gath", bufs=6) as gpool,
            tc.tile_pool(name="epil", bufs=8) as epool,
            tc.tile_pool(name="psum", bufs=2, space="PSUM") as pp,
            tc.tile_pool(name="dram", bufs=1, space="DRAM") as dpool,
        ):
            # ---- constants
            w1aug_sb = cpool.tile([fin, h + 2], bf, name="w1aug_sb")
            nc.sync.dma_start(w1aug_sb[:], w1aug)
            w2aug_sb = cpool.tile([h, c + 2], bf, name="w2aug_sb")
            nc.sync.dma_start(w2aug_sb[:], w2aug)
            b1aug_sb = cpool.tile([h + 2, 1], f32, name="b1aug_sb")
            nc.sync.dma_start(b1aug_sb[:], b1aug)
            b2aug_sb = cpool.tile([c + 2, 1], f32, name="b2aug_sb")
            nc.sync.dma_start(b2aug_sb[:], b2aug)
            iota_sb = cpool.tile([128, segcap], bf, name="iota_sb")
            nc.sync.dma_start(iota_sb[:], iota)
            ident_sb = cpool.tile([128, 128], bf, name="ident_sb")
            nc.sync.dma_start(ident_sb[:], ident)
            ones_sb = cpool.tile([1, 128], bf, name="ones_sb")
            nc.vector.memset(ones_sb[:], 1.0)
            srcg_sb = cpool.tile([128, T * k], i32, name="srcg_sb")
            nc.sync.dma_start(srcg_sb[:], srcg)
            srcg0_sb = cpool.tile([128, T], i32, name="srcg0_sb")
            nc.sync.dma_start(srcg0_sb[:], srcg0)
            segid_sb = cpool.tile([128, T * k], bf, name="segid_sb")
            nc.sync.dma_start(segid_sb[:], segid)
            outmap_sb = cpool.tile([128, ncall], i32, name="outmap_sb")
            nc.sync.dma_start(outmap_sb[:], outmap)

            # ---- internal DRAM (slot-ordered stages)
            h1s = dpool.tile([nslot, h + 2], bf, name="h1s")
            h1f = dpool.tile([N_CORES * nslot, h + 2], bf, name="h1f",
                             addr_space="Shared")
            g2s = dpool.tile([nslot, c + 2], bf, name="g2s")
            g2f = dpool.tile([N_CORES * nslot, c + 2], bf, name="g2f",
                             addr_space="Shared")
            fins = dpool.tile([nslot, c], f32, name="fins")

            # ---- phase 0: h1aug by slot, + dense a_dst column
            Hh = nslot // 2
            PH = 512
            for o in range(0, nslot, PH):
                w = min(PH, nslot - o)
                xt = ppool.tile([fin, PH], f32, name="xt")
                nc.sync.dma_start(xt[:, 0:w], xT[:, o:o + w])
                xtb = ppool.tile([fin, PH], bf, name="xtb")
                nc.vector.tensor_copy(xtb[:, 0:w], xt[:, 0:w])
                psH = pp.tile([h + 2, PH], f32, name="psH", tag="pA")
                nc.tensor.matmul(psH[:, 0:w], lhsT=w1aug_sb[:],
                                 rhs=xtb[:, 0:w], start=True, stop=True)
                h1t = ppool.tile([h + 2, PH], bf, name="h1t")
                nc.scalar.activation(h1t[:, 0:w], psH[:, 0:w], AF.Identity,
                                     bias=b1aug_sb[:])
                for q in range(0, w, 128):
                    wq = min(128, w - q)
                    psT0 = pp.tile([128, h + 2], bf, name="psT0", tag="pT")
                    nc.tensor.transpose(
                        psT0[0:wq, :], in_=h1t[:, q:q + wq],
                        identity=ident_sb[0:h + 2, 0:h + 2])
                    h1r = ppool.tile([128, h + 2], bf, name="h1r")
                    nc.vector.tensor_copy(h1r[0:wq, :], psT0[0:wq, :])
                    nc.sync.dma_start(h1s[o + q:o + q + wq, :], h1r[0:wq, :])

            nc.gpsimd.collective_compute(
                "AllGather", mybir.AluOpType.bypass,
                replica_groups=[list(range(N_CORES))],
                ins=[h1s[:]], outs=[h1f[:]],
            )

            # ---- edge phases
            def edge_layer(table, fdim, stage, last, hook=None):
                """gathers rw = fdim+1 elems [feat | a_src] per edge from
                table [8*nslot, fdim+2]; a_dst comes from the local stage
                table's a_dst column via a strided per-super DMA."""
                rw = fdim + 1
                for S in range(nsup):
                    c0 = sup * k * S
                    rows = gpool.tile([128, sup * k * rw], bf,
                                      name=f"rows{last}")
                    for i in range(sup):
                        t = S * sup + i
                        nc.sync.dma_start(
                            rows[0:segcap, i * k * rw:i * k * rw + rw],
                            stage[t * segcap:(t + 1) * segcap, 0:rw])
                    for ch in range(sup * k):
                        if ch % k == 0:
                            t = S * sup + ch // k
                            nc.gpsimd.indirect_dma_start(
                                out=rows[segcap:128, ch * rw:(ch + 1) * rw],
                                out_offset=None, in_=table[:],
                                in_offset=bass.IndirectOffsetOnAxis(
                                    ap=srcg0_sb[0:128 - segcap, t:t + 1],
                                    axis=0),
                                element_offset=0,
                            )
                        else:
                            nc.gpsimd.indirect_dma_start(
                                out=rows[:, ch * rw:(ch + 1) * rw],
                                out_offset=None, in_=table[:],
                                in_offset=bass.IndirectOffsetOnAxis(
                                    ap=srcg_sb[:, c0 + ch:c0 + ch + 1],
                                    axis=0),
                                element_offset=0,
                            )
                    # broadcast this super-tile's a_dst slots to all partitions
                    adrow = gpool.tile([1, scs], bf, name=f"adrow{last}")
                    nc.sync.dma_start(
                        adrow[:],
                        stage[S * scs:(S + 1) * scs,
                              fdim + 1:fdim + 2].transpose([1, 0]))
                    psB = pp.tile([128, scs], f32, name=f"psB{last}",
                                   tag="pC")
                    nc.tensor.matmul(psB[:], lhsT=ones_sb[:],
                                     rhs=adrow[:], start=True, stop=True)
                    adflat = gpool.tile([128, scs], bf, name=f"adf{last}")
                    nc.vector.tensor_copy(adflat[:], psB[:])
                    # one-hot of segids, a_dst expansion, attention weights
                    ohs = gpool.tile([128, sup * k * segcap], bf,
                                     name=f"ohs{last}")
                    ov = ohs[:].rearrange("p (e s) -> p e s", s=segcap)
                    nc.vector.tensor_tensor(
                        out=ov,
                        in0=iota_sb[:].unsqueeze(1).broadcast_to(
                            [128, sup * k, segcap]),
                        in1=segid_sb[:, c0:c0 + sup * k].unsqueeze(
                            2).broadcast_to([128, sup * k, segcap]),
                        op=AO.is_equal)
                    tmp = gpool.tile([128, sup * k * segcap], bf,
                                     name=f"tmp{last}")
                    nc.vector.tensor_tensor(
                        out=tmp[:].rearrange("p (t q s) -> p t q s", q=k,
                                             s=segcap),
                        in0=ohs[:].rearrange("p (t q s) -> p t q s", q=k,
                                             s=segcap),
                        in1=adflat[:].rearrange(
                            "p (t s) -> p t s", s=segcap).unsqueeze(
                            2).broadcast_to([128, sup, k, segcap]),
                        op=AO.mult)
                    ade = gpool.tile([128, sup * k], f32, name=f"ade{last}")
                    nc.vector.tensor_reduce(
                        ade[:], tmp[:].rearrange("p (e s) -> p e s",
                                                 s=segcap),
                        axis=AX.X, op=AO.add)
                    rv = rows[:].rearrange("p (e f) -> p e f", f=rw)
                    es = gpool.tile([128, sup * k], f32, name=f"es{last}")
                    nc.vector.tensor_tensor(out=es[:], in0=rv[:, :, rw - 1],
                                            in1=ade[:], op=AO.add)
                    e2 = gpool.tile([128, sup * k], f32, name=f"e2{last}")
                    nc.vector.tensor_scalar_mul(e2[:], es[:], 0.2)
                    nc.vector.tensor_tensor(out=es[:], in0=es[:], in1=e2[:],
                                            op=AO.max)
                    ps = gpool.tile([128, sup * k], bf, name=f"ps{last}")
                    nc.scalar.activation(ps[:], es[:], AF.Exp)
                    nc.vector.memset(rv[:, :, rw - 1], 1.0)
                    nc.vector.tensor_tensor(
                        out=ov, in0=ov,
                        in1=ps[:].unsqueeze(2).broadcast_to(
                            [128, sup * k, segcap]),
                        op=AO.mult)
                    for i in range(sup):
                        t = S * sup + i
                        psA = pp.tile([segcap, rw], f32, name=f"psA{last}",
                                      tag="pA")
                        for kk in range(k):
                            ch = i * k + kk
                            nc.tensor.matmul(
                                psA[:],
                                lhsT=ohs[:, ch * segcap:(ch + 1) * segcap],
                                rhs=rows[:, ch * rw:(ch + 1) * rw],
                                start=(kk == 0), stop=(kk == k - 1))
                        den = epool.tile([segcap, 1], f32, name=f"den{last}")
                        nc.vector.tensor_scalar(
                            den[:], psA[:, fdim:fdim + 1], 1e-30, None,
                            op0=AO.max)
                        rcp = epool.tile([segcap, 1], f32, name=f"rcp{last}")
                        nc.vector.reciprocal(rcp[:], den[:])
                        if not last:
                            h2n = epool.tile([segcap, fdim], bf, name="h2n")
                            nc.vector.tensor_scalar(
                                h2n[:], psA[:, 0:fdim], rcp[:], 0.0,
                                op0=AO.mult, op1=AO.max)
                            psT = pp.tile([fdim, segcap], bf, name="psT",
                                          tag="pT")
                            nc.tensor.transpose(
                                psT[:], in_=h2n[:],
                                identity=ident_sb[0:segcap, 0:segcap])
                            h2nT = epool.tile([fdim, segcap], bf, name="h2nT")
                            nc.vector.tensor_copy(h2nT[:], psT[:])
                            psC = pp.tile([c + 2, segcap], f32, name="psC",
                                          tag="pC")
                            nc.tensor.matmul(psC[:], lhsT=w2aug_sb[:],
                                             rhs=h2nT[:], start=True,
                                             stop=True)
                            c1 = epool.tile([c + 2, segcap], bf, name="c1")
                            nc.scalar.activation(c1[:], psC[:], AF.Identity,
                                                 bias=b2aug_sb[:])
                            psD = pp.tile([segcap, c + 2], bf, name="psD",
                                          tag="pT")
                            nc.tensor.transpose(
                                psD[:], in_=c1[:],
                                identity=ident_sb[0:c + 2, 0:c + 2])
                            orow = epool.tile([segcap, c + 2], bf,
                                              name="orow")
                            nc.vector.tensor_copy(orow[:], psD[:])
                            nc.sync.dma_start(
                                g2s[t * segcap:(t + 1) * segcap, :], orow[:])
                        else:
                            fin_t = epool.tile([segcap, c], f32, name="fin")
                            nc.vector.tensor_scalar(
                                fin_t[:], psA[:, 0:c], rcp[:], None,
                                op0=AO.mult)
                            nc.sync.dma_start(
                                fins[t * segcap:(t + 1) * segcap, :],
                                fin_t[:])
                    if hook is not None:
                        hook(S)

            edge_layer(h1f, h, h1s, last=False)
            nc.gpsimd.collective_compute(
                "AllGather", mybir.AluOpType.bypass,
                replica_groups=[list(range(N_CORES))],
                ins=[g2s[:]], outs=[g2f[:]],
            )

            # ---- slot -> node conversion, interleaved into the L2 loop.
            # slotmap is monotone and slots >= nodes, so the slots of nodes
            # [0, 128(j+1)) all lie below nslot - nshard + 128(j+1): call j
            # may run once the stage rows below that bound are final.
            conv_done = [0]

            def emit_conv(upto):
                while conv_done[0] < ncall:
                    j = conv_done[0]
                    bound = min(nslot, nslot - nshard + 128 * (j + 1))
                    if bound > upto:
                        break
                    w = min(128, nshard - j * 128)
                    fj = epool.tile([128, c], f32, name="fj")
                    nc.gpsimd.indirect_dma_start(
                        out=fj[0:w, :], out_offset=None,
                        in_=fins[0:bound, :],
                        in_offset=bass.IndirectOffsetOnAxis(
                            ap=outmap_sb[0:w, j:j + 1], axis=0),
                        element_offset=0,
                    )
                    nc.sync.dma_start(out2[j * 128:j * 128 + w, :],
                                      fj[0:w, :])
                    conv_done[0] += 1

            def l2_hook(S):
                emit_conv((S + 1) * sup * segcap)

            edge_layer(g2f, c, g2s, last=True, hook=l2_hook)
            emit_conv(nslot)

    _compress_deps(nc)
    nc.compile()
    return nc


# ------------------------------------------------------------------ interface
def make_inmaps(inputs, cfg):
    x = np.ascontiguousarray(np.asarray(inputs["x"], np.float32))
    W1 = np.asarray(inputs["W1"], np.float32)
    as1 = np.asarray(inputs["att_src1"], np.float32)
    ad1 = np.asarray(inputs["att_dst1"], np.float32)
    b1 = np.asarray(inputs["b1"], np.float32)
    W2 = np.asarray(inputs["W2"], np.float32)
    as2 = np.asarray(inputs["att_src2"], np.float32)
    ad2 = np.asarray(inputs["att_dst2"], np.float32)
    b2 = np.asarray(inputs["b2"], np.float32)
    cores, T = preprocess(np.asarray(inputs["edge_index"]), cfg)
    w1aug = np.concatenate([W1, (W1 @ as1)[:, None], (W1 @ ad1)[:, None]], 1)
    w2aug = np.concatenate([W2, (W2 @ as2)[:, None], (W2 @ ad2)[:, None]], 1)
    b1aug = np.concatenate([b1, [0.0, 0.0]]).astype(np.float32)[:, None]
    b2aug = np.concatenate([b2, [0.0, 0.0]]).astype(np.float32)[:, None]
    nshard, segcap = cfg["nshard"], cfg["segcap"]
    iota = np.broadcast_to(np.arange(segcap, dtype=np.float32),
                           (128, segcap)).astype(BF16)
    ident = np.eye(128, dtype=np.float32).astype(BF16)
    in_maps = []
    for cidx in range(N_CORES):
        xs = x[cidx * nshard:(cidx + 1) * nshard]      # [nshard, fin]
        xslot = xs[cores[cidx]["slotnode"]]            # [nslot, fin]
        in_maps.append(dict(
            xT=np.ascontiguousarray(xslot.T),
            w1aug=np.ascontiguousarray(w1aug.astype(BF16)),
            w2aug=np.ascontiguousarray(w2aug.astype(BF16)),
            b1aug=np.ascontiguousarray(b1aug),
            b2aug=np.ascontiguousarray(b2aug),
            iota=np.ascontiguousarray(iota),
            ident=np.ascontiguousarray(ident),
            srcg=cores[cidx]["srcg"],
            srcg0=cores[cidx]["srcg0"],
            segid=cores[cidx]["segid"],
            outmap=cores[cidx]["outmap"],
        ))
    return in_maps, T


def kernel(**inputs):
    from concourse import bass_utils

    cfg = dict(DEF_CFG)
    in_maps, T = make_inmaps(inputs, cfg)
    nc = build_program(cfg, T)
    res = bass_utils.run_bass_kernel_spmd(
        nc, in_maps, core_ids=list(range(N_CORES)))
    out = np.concatenate([res.results[c]["out2"] for c in range(N_CORES)], 0)
    return out.astype(np.float32)
